# revision 2
# baseline (speedup 1.0000x reference)
"""Expert-parallel MoE FFN kernel for Trainium2 (8 NeuronCores).

Problem: inputs [B=2, E=8, C=8192, H=512], per-expert FFN
    h   = gelu_tanh(x_e @ w1_e + b1_e)        (w1: [E, H, F=2048])
    out = h @ w2_e + b2_e                     (w2: [E, F, H])

Sharding: expert-parallel — core e owns expert e's tokens [B*C, H] and
weights; no cross-core communication.

Per-core dataflow (matmuls in float32r = full-rate fp32 on the PE):
    x tile [128 tok, 512 H]  --PE transpose-->  xT [H-part, tok]
    GEMM1: hT[f,:] += w1[hk, f-chunk].T @ xT[hk, :]   (lhsT = w1, natural layout)
    gelu on ACT during PSUM->SBUF eviction (bias = b1 per-partition)
    GEMM2: out[tok,:] += hT[fk, tok-chunk].T @ w2[fk, :] (lhsT = hT, rhs = w2 natural)
    b2 add on DVE during PSUM->SBUF eviction

FP32r discipline: the BIR verifier requires every producer of an f32r
matmul input to round its output to f32r. xT and hT are produced by
DVE/ACT ops writing into f32r tiles (rounding happens on the write);
weights are DMA'd as raw bytes into f32r tiles ("dma" mode) or staged
through f32 tiles + a rounding DVE copy ("stage" mode).
"""

import numpy as np

_B, _E, _C, _H, _F = 2, 8, 8192, 512, 2048
_TOK = _B * _C  # 16384 tokens per expert
_P = 128
_T = 512  # tokens per macro tile

_MM_MODE = "f32r_dma"  # "f32r_dma" | "f32r_stage" | "f32"

_cache = {}


def build_nc(tok=_TOK, act_name="Gelu_apprx_tanh", n_devices=_E, mm_mode=_MM_MODE):
    import concourse.mybir as mybir
    import concourse.tile as tile
    from concourse import bacc
    from concourse.masks import make_identity

    H, F, P, T = _H, _F, _P, _T
    HK, FK = H // P, F // P  # 4, 16 contraction chunks
    NJ = T // P  # 4 token sub-blocks per macro tile
    NM = tok // T  # macro tiles
    f32 = mybir.dt.float32
    f32r = mybir.dt.float32r
    mmdt = f32 if mm_mode == "f32" else f32r
    act = getattr(mybir.ActivationFunctionType, act_name)

    nc = bacc.Bacc("TRN2", debug=False, target_bir_lowering=False,
                   num_devices=n_devices)
    wdt = f32r if mm_mode == "f32r_dma" else f32
    x = nc.dram_tensor("x", [tok, H], f32, kind="ExternalInput").ap()
    w1 = nc.dram_tensor("w1", [H, F], wdt, kind="ExternalInput").ap()
    b1 = nc.dram_tensor("b1", [F], f32, kind="ExternalInput").ap()
    w2 = nc.dram_tensor("w2", [F, H], wdt, kind="ExternalInput").ap()
    b2 = nc.dram_tensor("b2", [H], f32, kind="ExternalInput").ap()
    out = nc.dram_tensor("out", [tok, H], f32, kind="ExternalOutput").ap()

    with tile.TileContext(nc) as tc:
        with (
            tc.tile_pool(name="const", bufs=1) as const,
            tc.tile_pool(name="stage", bufs=1) as stage,
            tc.tile_pool(name="xin", bufs=2) as xin_pool,
            tc.tile_pool(name="xt", bufs=2) as xt_pool,
            tc.tile_pool(name="ht", bufs=1) as ht_pool,
            tc.tile_pool(name="obuf", bufs=2) as o_pool,
            tc.tile_pool(name="pst", bufs=2, space="PSUM") as psT,
            tc.tile_pool(name="ps1", bufs=3, space="PSUM") as ps1,
            tc.tile_pool(name="ps2", bufs=3, space="PSUM") as ps2,
        ):
            # --- weights / constants, resident in SBUF for the whole kernel
            w1_sb = const.tile([P, HK, F], mmdt)
            w2_sb = const.tile([P, FK, H], mmdt)
            if mm_mode == "f32r_stage":
                w1st = stage.tile([P, HK * F], f32, tag="wst")
                nc.sync.dma_start(
                    w1st[:], w1.rearrange("(hk p) f -> p (hk f)", p=P))
                nc.vector.tensor_copy(
                    w1_sb.rearrange("p hk f -> p (hk f)"), w1st[:])
                w2st = stage.tile([P, FK * H], f32, tag="wst")
                nc.sync.dma_start(
                    w2st[:], w2.rearrange("(fk p) h -> p (fk h)", p=P))
                nc.vector.tensor_copy(
                    w2_sb.rearrange("p fk h -> p (fk h)"), w2st[:])
            else:
                nc.sync.dma_start(
                    w1_sb[:], w1.rearrange("(hk p) f -> p hk f", p=P))
                nc.sync.dma_start(
                    w2_sb[:], w2.rearrange("(fk p) h -> p fk h", p=P))
            b1_sb = const.tile([P, FK], f32)
            nc.sync.dma_start(b1_sb[:], b1.rearrange("(fk p) -> p fk", p=P))
            b2_row = const.tile([1, H], f32)
            nc.sync.dma_start(b2_row[:], b2[None, :])
            ones = const.tile([1, P], f32)
            nc.any.memset(ones[:], 1.0)
            ident = const.tile([P, P], f32)
            make_identity(nc, ident[:])
            # broadcast b2 across all 128 partitions via a K=1 matmul
            b2_bc = const.tile([P, H], f32)
            ps_b2 = ps2.tile([P, H], f32, tag="po")
            nc.tensor.matmul(ps_b2[:], ones[:], b2_row[:], start=True, stop=True)
            nc.vector.tensor_copy(b2_bc[:], ps_b2[:])

            for m in range(NM):
                r = m * T
                xbig = xin_pool.tile([P, NJ, H], f32)
                nc.sync.dma_start(
                    xbig[:], x[r:r + T, :].rearrange("(j p) h -> p j h", p=P))

                # transpose x tile: [tok, H] -> xT [H-chunk partitions, tok]
                xt = xt_pool.tile([P, HK, T], mmdt)
                for j in range(NJ):
                    for hk in range(HK):
                        pt = psT.tile([P, P], f32)
                        nc.tensor.transpose(
                            pt[:], xbig[:, j, hk * P:(hk + 1) * P], ident[:])
                        nc.vector.tensor_copy(xt[:, hk, j * P:(j + 1) * P], pt[:])

                # GEMM1 + gelu: hT [F-chunk partitions, tok]
                ht = ht_pool.tile([P, FK, T], mmdt)
                for fm in range(FK):
                    ph = ps1.tile([P, T], f32)
                    for hk in range(HK):
                        nc.tensor.matmul(
                            ph[:],
                            w1_sb[:, hk, fm * P:(fm + 1) * P],
                            xt[:, hk, :],
                            start=(hk == 0),
                            stop=(hk == HK - 1),
                        )
                    nc.scalar.activation(
                        ht[:, fm, :], ph[:], act, bias=b1_sb[:, fm:fm + 1])

                # GEMM2 + b2: out tile [tok partitions, H]
                obig = o_pool.tile([P, NJ, H], f32)
                for j in range(NJ):
                    po = ps2.tile([P, H], f32, tag="po")
                    for fk in range(FK):
                        nc.tensor.matmul(
                            po[:],
                            ht[:, fk, j * P:(j + 1) * P],
                            w2_sb[:, fk, :],
                            start=(fk == 0),
                            stop=(fk == FK - 1),
                        )
                    nc.vector.tensor_add(obig[:, j, :], po[:], b2_bc[:])
                nc.sync.dma_start(
                    out[r:r + T, :].rearrange("(j p) h -> p j h", p=P), obig[:])

    nc.compile()
    return nc


def kernel(inputs, w1, b1, w2, b2):
    from concourse.bass_utils import run_bass_kernel_spmd

    inputs = np.asarray(inputs, dtype=np.float32)
    w1 = np.asarray(w1, dtype=np.float32)
    b1 = np.asarray(b1, dtype=np.float32)
    w2 = np.asarray(w2, dtype=np.float32)
    b2 = np.asarray(b2, dtype=np.float32)

    B, E, C, H = inputs.shape
    tok = B * C
    # [B, E, C, H] -> per-expert token matrix [E, B*C, H]
    x = np.ascontiguousarray(inputs.transpose(1, 0, 2, 3).reshape(E, tok, H))

    if "nc" not in _cache:
        _cache["nc"] = build_nc()
    nc = _cache["nc"]

    in_maps = [
        {
            "x": x[e],
            "w1": np.ascontiguousarray(w1[e]),
            "b1": np.ascontiguousarray(b1[e]),
            "w2": np.ascontiguousarray(w2[e]),
            "b2": np.ascontiguousarray(b2[e]),
        }
        for e in range(E)
    ]
    res = run_bass_kernel_spmd(nc, in_maps, core_ids=list(range(E)))
    o = np.stack([res.results[e]["out"] for e in range(E)])  # [E, tok, H]
    return np.ascontiguousarray(
        o.reshape(E, B, C, H).transpose(1, 0, 2, 3))
